# revision 27
# baseline (speedup 1.0000x reference)
"""Trainium2 Bass kernel for nn_AMIPRouterInference (gnn_message_passing).

Strategy
--------
Algebraic restructure of the reference (~515 GFLOP -> ~52 GFLOP):
  * cond @ W1 splits into h_anc @ W1a + h_ctr @ W1b, each computed once per
    token (not once per window pair):  u = h @ W1b, v = h @ W1a.
  * The attention combine over the +-r window commutes with the W2 matmul:
    g = sum_n cw_n * gelu(v[l+off_n] + u[l]);  delta = (w * g) @ W2 + w @ b2.

Sharding: pure data-parallel over the B*L = 4096 tokens -> 512 tokens/core on
8 cores; the +-5 halo is baked into each core's input shard on the host, so no
collectives are needed.

Per-core layout: features-on-partitions (u/v as 16 chunks of [128, tokens]) so
window shifts along tokens are free-axis SBUF slices.  Even/odd phase copies of
v keep the bf16 DVE 2x alignment for shifted adds.

Key engine facts this schedule is built around:
  * DVE is the bottleneck engine (~165us of tensor_tensor at bf16 2x).
    Batched multi-row-AP adds keep the 2x packing when every row start is
    4B-aligned (hardware-verified).
  * PE clock is HAM-gated: 1.2 GHz cold, 2.4 GHz after ~3.4us of sustained
    activity; any >3.4us idle window re-throttles.  The delta-stage matmuls
    are paced per-combine through the back half so the post-combine(15)
    tail is only ~23 matmuls.
  * DMA issue costs ~650ns per descriptor on the in-order sync queue, so
    startup inputs are packed host-side into 5 large contiguous transfers.
  * Emission priority order: all 4 gram tiles + extracts, then the whole
    B phase (transposes/exps/cw+w round trips) BEFORE the first gelu, so
    the score path is never priority-starved and the exps share one ACT
    table residency.  The ready-heap Tile scheduler backfills PE with
    d-half matmuls while the extract chain drains on Vector.
"""

import sys

for _p in ("/opt/trn_rl_repo", "/root/.axon_site/_ro/trn_rl_repo"):
    if _p not in sys.path:
        sys.path.append(_p)

import numpy as np
import ml_dtypes

import bass_rust
import concourse.bacc as bacc
import concourse.mybir as mybir
import concourse.tile as tile
from concourse.bass_utils import run_bass_kernel_spmd

BF16 = ml_dtypes.bfloat16

# Problem constants (hardcoded per spec).
B, L, D = 2, 2048, 1024
K, D4, R = 8, 256, 5
NCORES = 8
T = (B * L) // NCORES          # tokens per core = 512
PADL = 16                      # left pad of the per-core token window
TP = T + 2 * PADL              # padded width = 544
NOFF = 2 * R                   # 10 window offsets
F = K * D4                     # 2048 fused expert features
NFC = F // 128                 # 16 feature chunks
NKC = D // 128                 # 8 contraction chunks
NTC = T // 128                 # 4 token tiles per core

# Offset processing order: even offsets first (read from v_even), then odd
# (read from v_odd, which holds v shifted left by one token).  Within each
# phase every slice start is an even element index -> 4-byte aligned, which
# keeps the DVE's bf16 2x packing for the batched multi-row adds.
OFF_ORDER = [-4, -2, 2, 4, -5, -3, -1, 1, 3, 5]

RUNWAY = 4                     # d_mm emitted this many fc ahead of combine

_SIM_SAFE_GELU = False         # CoreSim lacks Gelu; swap in Tanh for sim runs

_CACHE = {}


def _build_graph():
    fp32 = mybir.dt.float32
    bf16 = mybir.dt.bfloat16

    nc = bacc.Bacc("TRN2", target_bir_lowering=False, debug=False,
                   num_devices=NCORES)

    # ---- DRAM parameters (per-core shards; same shapes on every core).
    # Startup tensors are host-packed so each is ONE contiguous DMA.
    hP = nc.dram_tensor("hP", [128, NKC * TP], bf16, kind="ExternalInput")
    cP = nc.dram_tensor("cP", [128, 216], fp32, kind="ExternalInput")
    wrP = nc.dram_tensor("wrP", [128, NKC * K + 1], bf16,
                         kind="ExternalInput")
    validT = nc.dram_tensor("validT", [11, T], bf16, kind="ExternalInput")
    b2o = nc.dram_tensor("b2o", [NOFF, D + 2], bf16, kind="ExternalInput")
    w1ab = nc.dram_tensor("w1ab", [NFC, 128, 2 * D], bf16,
                          kind="ExternalInput")
    w2 = nc.dram_tensor("w2", [NFC, 128, D], bf16, kind="ExternalInput")
    out = nc.dram_tensor("out", [T, D], fp32, kind="ExternalOutput")

    AF = mybir.ActivationFunctionType
    OP = mybir.AluOpType

    def bc_ap(tile_, inner_rep, ncols):
        """[128, ncols] tile viewed as [128, ncols, inner_rep] via a step-0
        innermost dim (per-partition broadcast along the replicated axis)."""
        return bass_rust.AP(
            tensor=tile_[:].tensor, offset=0,
            ap=[[ncols, 128], [1, ncols], [0, inner_rep]])

    def rows_ap(tile_, off, ostep, ocnt, icnt):
        """Multi-row free AP: ocnt rows of icnt step-1 elements, row starts
        off, off+ostep, ...  (all starts must be 4B-aligned for bf16 2x)."""
        return bass_rust.AP(
            tensor=tile_[:].tensor, offset=off,
            ap=[[tile_[:].shape[1], 128], [ostep, ocnt], [1, icnt]])

    with tile.TileContext(nc) as tc:
        with (
            tc.tile_pool(name="const", bufs=1) as cpool,
            tc.tile_pool(name="hpool", bufs=1) as hpool,
            tc.tile_pool(name="w2pool", bufs=1) as w2pool,
            tc.tile_pool(name="w1pool", bufs=3) as w1pool,
            tc.tile_pool(name="small", bufs=2) as spool,
            tc.tile_pool(name="persist", bufs=1) as ppool,
            tc.tile_pool(name="uv", bufs=2) as uvpool,
            tc.tile_pool(name="big", bufs=2) as bigpool,
                        tc.tile_pool(name="tbuf", bufs=1) as qpool,
            tc.tile_pool(name="ppart", bufs=1) as partpool,
            tc.tile_pool(name="gout", bufs=1) as gpool,
            tc.tile_pool(name="opool", bufs=4) as opool,
            tc.tile_pool(name="dram", bufs=1, space="DRAM") as dpool,
            tc.tile_pool(name="ps_big", bufs=4, space="PSUM") as psb,
            tc.tile_pool(name="ps_vb", bufs=1, space="PSUM") as psvb,
            tc.tile_pool(name="ps_small", bufs=3, space="PSUM") as pss,
            # PSUM budget (8 banks): psb "m" 4 (u/va double buffer; the 4
            # banks are reused for delta preopens once stage D ends),
            # psvb "vb" 1, pss "s" 3 (logits, transposes, den, E1
            # transients, and the 3 held delta groups).
        ):
            # ---------------- packed startup loads ----------------
            # h split across the two HWDGE rings (sync + scalar) so the
            # load finishes ~3.5us earlier; everything downstream gates on
            # the first gram tile.
            h_m = hpool.tile([128, NKC * TP], bf16, tag="h")
            nc.sync.dma_start(h_m[:, :4 * TP], hP[:, :4 * TP])
            nc.scalar.dma_start(h_m[:, 4 * TP:], hP[:, 4 * TP:])
            c_m = cpool.tile([128, 216], fp32, tag="c")
            nc.sync.dma_start(c_m[:], cP[:])
            wr_m = cpool.tile([128, NKC * K + 1], bf16, tag="wr")
            nc.sync.dma_start(wr_m[:], wrP[:])
            validT_sb = cpool.tile([11, T], bf16, tag="validT")
            nc.sync.dma_start(validT_sb[:], validT[:])
            b2o_sb = cpool.tile([NOFF, D + 2], bf16, tag="b2o")
            nc.sync.dma_start(b2o_sb[:], b2o[:])

            h_sb = [h_m[:, kc * TP:(kc + 1) * TP] for kc in range(NKC)]
            ident_sb = c_m[:, 0:128]
            br_sb = c_m[:, 128:160]
            b1_sb = c_m[:, 160:176]
            vtok_sb = c_m[:, 176:216]
            wr_sb = [wr_m[:, kc * K:(kc + 1) * K] for kc in range(NKC)]
            ones11_sb = wr_m[0:11, NKC * K:NKC * K + 1]
            b2_sb = b2o_sb[0:K, 0:D]

            # fc0's W1 rides the (idle) SWDGE queue so it lands in
            # parallel with both HWDGE rings' h/const loads.
            w1_first = w1pool.tile([128, 2 * D], bf16, tag="w1")
            nc.gpsimd.dma_start(w1_first[:], w1ab[0])

            # Hoist both ACT table loads (exp + gelu sets, ~1.3us each) into
            # the h-DMA shadow via 1-column dummy activations.
            warm = spool.tile([1, 1], fp32, tag="warm")
            nc.scalar.activation(warm[:], ident_sb[0:1, 0:1], AF.Exp)

            # persistent transposed score & router weights (bf16)
            cwT_bf = ppool.tile([11, T], bf16, tag="cwT")
            wT_bf = ppool.tile([K, T], bf16, tag="wT")
            # cw_bc in SORTED offset order (block n = offset n-5, 11 blocks
            # incl the always-invalid center), so the DRAM round trip needs
            # no permutation; the combine mult slices it per phase.
            cw_bc = gpool.tile([128, 11 * 512], bf16, tag="cw_bc")
            w_bc_all = gpool.tile([128, K * 512], bf16, tag="w_bc_all")

            # ------------- stage A/B/C: scores, cw, router w -------------
            # Phase A: gram/router matmuls.  The diagonal extraction rides
            # DMA instead of DVE: evacuate the gram tile to SBUF fp16, dump
            # it to DRAM flat with row pitch 139, and read the band
            # s[p, n] = flat[p*140 + n] back as an [11, 128] strided load.
            lg_all = spool.tile([128, NTC * K], fp32, tag="lg_all")
            sT11 = ppool.tile([11, T], mybir.dt.float16, tag="sT11")

            def phase_gram(tci):
                c0 = PADL + tci * 128
                g_ps = psb.tile([128, 512], fp32, tag="m")
                lg_ps = pss.tile([128, K], fp32, tag="s")
                for kc in range(NKC):
                    st = (kc == 0)
                    sp = (kc == NKC - 1)
                    nc.tensor.matmul(g_ps[:, :138],
                                     h_sb[kc][:, c0:c0 + 128],
                                     h_sb[kc][:, c0 - 5:c0 + 133],
                                     start=st, stop=sp)
                    nc.tensor.matmul(lg_ps[:],
                                     h_sb[kc][:, c0:c0 + 128],
                                     wr_sb[kc],
                                     start=st, stop=sp)
                g16 = spool.tile([128, 139], mybir.dt.float16,
                                 tag=f"g16_{tci}")
                nc.gpsimd.memset(g16[:, 138:139], 0.0)
                nc.scalar.copy(g16[:, 0:138], g_ps[:, :138])
                gdump = dpool.tile([1, 128 * 139], mybir.dt.float16,
                                   tag=f"gd{tci}")
                eng = nc.sync if tci % 2 == 0 else nc.scalar
                eng.dma_start(gdump[:], g16[:])
                diag_src = bass_rust.AP(
                    tensor=gdump[:].tensor, offset=0,
                    ap=[[1, 11], [140, 128]])
                eng.dma_start(sT11[:, tci * 128:(tci + 1) * 128], diag_src)
                nc.scalar.copy(lg_all[:, tci * K:(tci + 1) * K], lg_ps[:])

            # Phase B1: exp the score band (scale 1/sqrt(D) folded into the
            # activation), mask with validity (center row forced 0), and
            # broadcast the UNNORMALIZED cw = exp(s)*valid to all partitions
            # via one DRAM round trip.  The softmax 1/den lands at the very
            # end as a per-token scale on the delta close.  No max-shift is
            # needed: scores are O(1) so exp() cannot over/underflow.
            def phase_b1():
                evT = ppool.tile([11, T], fp32, tag="evT")
                nc.scalar.activation(evT[:], sT11[:], AF.Exp,
                                     scale=1.0 / 32.0)
                nc.vector.tensor_mul(cwT_bf[:], evT[:], validT_sb[:])
                # scalar ring: the sync ring carries the bulky w1/w2 loads,
                # which would head-of-line block this latency-critical pair.
                cw_dram = dpool.tile([1, 11 * T], bf16, tag="cw_dram")
                nc.scalar.dma_start(cw_dram[:], cwT_bf[:])
                nc.scalar.dma_start(cw_bc[:],
                                    cw_dram[:].partition_broadcast(128))

            rdenT = ppool.tile([128, NTC], fp32, tag="rdenT")
            fT = ppool.tile([128, NTC], fp32, tag="fT")
            wplT_bf = ppool.tile([K, T], bf16, tag="wplT")
            w_pl = ppool.tile([128, NTC * K], fp32, tag="w_pl")

            we = spool.tile([128, NTC * K], fp32, tag="we")

            def phase_b2a_pre():
                """Router softmax up to the exp."""
                nc.vector.tensor_add(lg_all[:], lg_all[:], br_sb)
                wmx = spool.tile([128, NTC], fp32, tag="wmx")
                lg3 = bass_rust.AP(tensor=lg_all[:].tensor, offset=0,
                                   ap=[[NTC * K, 128], [K, NTC], [1, K]])
                nc.vector.reduce_max(wmx[:], lg3, mybir.AxisListType.X)
                nc.vector.tensor_sub(we[:], lg_all[:], bc_ap(wmx, K, NTC))
                nc.scalar.activation(we[:], we[:], AF.Exp)

            def phase_b2a_post():
                """Post-exp half: w softmax, transpose, broadcast.  Emitted
                before the first gelu so the w_dram round trip isn't queued
                behind 5.5us ACT gelus."""
                wsum = spool.tile([128, NTC], fp32, tag="wsum")
                we3 = bass_rust.AP(tensor=we[:].tensor, offset=0,
                                   ap=[[NTC * K, 128], [K, NTC], [1, K]])
                nc.vector.reduce_sum(wsum[:], we3, mybir.AxisListType.X)
                rws = spool.tile([128, NTC], fp32, tag="rws")
                nc.vector.reciprocal(rws[:], wsum[:])
                nc.vector.tensor_mul(w_pl[:], we[:], bc_ap(rws, K, NTC))
                for tci in range(NTC):
                    wpT_ps = pss.tile([K, 128], fp32, tag="s")
                    nc.tensor.transpose(wpT_ps[:],
                                        w_pl[:, tci * K:(tci + 1) * K],
                                        ident_sb)
                    nc.scalar.copy(wplT_bf[:, tci * 128:(tci + 1) * 128],
                                   wpT_ps[:])
                w_dram = dpool.tile([1, K * T], bf16, tag="w_dram")
                nc.scalar.dma_start(w_dram[:], wplT_bf[:])
                nc.scalar.dma_start(w_bc_all[:],
                                    w_dram[:].partition_broadcast(128))

            weff3 = spool.tile([128, NTC * K], fp32, tag="weff3")

            def phase_b2b_early():
                """Denominator per token: one PE ones-matmul over the 11 cw
                rows, then 4 tiny transposes into token-per-partition layout
                for the close-time reciprocal scale."""
                den_ps = pss.tile([1, T], fp32, tag="s")
                nc.tensor.matmul(den_ps[:], ones11_sb, cwT_bf[:],
                                 start=True, stop=True)
                den_sb = spool.tile([1, T], fp32, tag="den_sb")
                nc.scalar.copy(den_sb[:], den_ps[:])
                dent_ps = pss.tile([128, NTC], fp32, tag="s")
                for tci in range(NTC):
                    # [1,128] -> [128,1] flip: contraction-1 matmul against
                    # the identity's single 1.0 cell.
                    nc.tensor.matmul(
                        dent_ps[:, tci:tci + 1],
                        den_sb[0:1, tci * 128:(tci + 1) * 128],
                        ident_sb[0:1, 0:1], start=True, stop=True)
                nc.scalar.copy(fT[:], dent_ps[:])
                dene_t = ppool.tile([128, NTC], fp32, tag="dene_t")
                nc.vector.tensor_scalar_add(dene_t[:], fT[:], 1e-30)
                nc.vector.reciprocal(rdenT[:], dene_t[:])
                # b2-path weights: w * raw_den (so the final 1/den scale on
                # the delta close reproduces w * sum_cw exactly)
                for tci in range(NTC):
                    nc.vector.tensor_scalar_mul(
                        weff3[:, tci * K:(tci + 1) * K],
                        w_pl[:, tci * K:(tci + 1) * K], fT[:, tci:tci + 1])

            def phase_b2b_late():
                """Tiny weff transposes; deferred so they never head-of-line
                block the PE queue while waiting on weff3."""
                for tci in range(NTC):
                    weT_ps = pss.tile([K, 128], fp32, tag="s")
                    nc.tensor.transpose(weT_ps[:],
                                        weff3[:, tci * K:(tci + 1) * K],
                                        ident_sb)
                    nc.scalar.copy(wT_bf[:, tci * 128:(tci + 1) * 128],
                                   weT_ps[:])

            # ------------- stage D: u/v matmuls + gelu combine -------------
            # fc are processed in PAIRS: both halves of pair p share expert
            # k=p, so the cw/w broadcasts extend across the pair via step-0
            # AP dims for free, and every DVE op doubles its width (halving
            # per-instruction overhead).  b1 is folded into the u evacuation
            # so the pair gelu needs no per-fc bias.
            g_sb = [None] * NFC
            tmp_sb = {}
            NP = NFC // 2

            def ap4(tile_, off, dims):
                """Free AP with the partition dim prepended; dims is a list
                of [step, count] free dims (outermost first)."""
                return bass_rust.AP(
                    tensor=tile_[:].tensor, offset=off,
                    ap=[[tile_[:].shape[1], 128]] + dims)

            def d_mm_half(fc, w1_t, u2, vev2, vod2, half):
                u_ps = psb.tile([128, 512], fp32, tag="m")
                va_ps = psb.tile([128, 512], fp32, tag="m")
                vb_ps = psvb.tile([128, 48], fp32, tag="vb")
                for kc in range(NKC):
                    st = (kc == 0)
                    sp = (kc == NKC - 1)
                    lhs_b = w1_t[:, kc * 128:(kc + 1) * 128]
                    lhs_a = w1_t[:, D + kc * 128:D + (kc + 1) * 128]
                    nc.tensor.matmul(u_ps[:], lhs_b,
                                     h_sb[kc][:, PADL:PADL + 512],
                                     start=st, stop=sp)
                    nc.tensor.matmul(va_ps[:], lhs_a,
                                     h_sb[kc][:, 0:512],
                                     start=st, stop=sp)
                    nc.tensor.matmul(vb_ps[:], lhs_a,
                                     h_sb[kc][:, 496:544],
                                     start=st, stop=sp)
                o_u = half * 512
                o_v = half * TP
                nc.scalar.activation(u2[:, o_u:o_u + 512], u_ps[:],
                                     AF.Identity, bias=b1_sb[:, fc:fc + 1])
                nc.scalar.copy(vev2[:, o_v:o_v + 512], va_ps[:])
                nc.scalar.copy(vev2[:, o_v + 512:o_v + 544], vb_ps[:, 16:48])
                nc.scalar.copy(vod2[:, o_v:o_v + 511], va_ps[:, 1:512])
                nc.scalar.copy(vod2[:, o_v + 511:o_v + 543], vb_ps[:, 16:48])

            def stage_d_gelu(p):
                tmp = tmp_sb[p]
                nc.scalar.activation(tmp[:], tmp[:],
                                     AF.Tanh if _SIM_SAFE_GELU else AF.Gelu)

            uv_sb = {}

            def stage_d_half(fc, w1_pre=None):
                p, half = fc // 2, fc % 2
                if half == 0:
                    u2 = uvpool.tile([128, 1024], bf16, tag="u")
                    vev2 = uvpool.tile([128, 2 * TP], bf16, tag="v_ev")
                    vod2 = uvpool.tile([128, 2 * TP], bf16, tag="v_od")
                    uv_sb[p] = (u2, vev2, vod2)
                u2, vev2, vod2 = uv_sb[p]
                if w1_pre is not None:
                    w1_t = w1_pre
                else:
                    w1_t = w1pool.tile([128, 2 * D], bf16, tag="w1")
                    nc.sync.dma_start(w1_t[:], w1ab[fc])
                d_mm_half(fc, w1_t, u2, vev2, vod2, half)

            def stage_d_adds(p, emit_gelu=True, chunked=False):
                u2, vev2, vod2 = uv_sb[p]
                tmp = bigpool.tile([128, 2 * NOFF * 512], bf16, tag="tmp")
                # Batched shifted adds across both halves (3 free dims; all
                # row starts stay 4B-aligned so bf16 2x packing holds).
                # Layout per half (matches OFF_ORDER):
                #   [0:1024)    offs -4,-2   [1024:2048) offs 2,4
                #   [2048:5120) odd offs -5..5 from v_od
                tmp_sb[p] = tmp

                def gelu_chunk(off, w):
                    nc.scalar.activation(
                        ap4(tmp, off, [[5120, 2], [1, w]]),
                        ap4(tmp, off, [[5120, 2], [1, w]]),
                        AF.Tanh if _SIM_SAFE_GELU else AF.Gelu)

                nc.vector.tensor_add(
                    ap4(tmp, 0, [[5120, 2], [512, 2], [1, 512]]),
                    ap4(vev2, PADL - 4, [[TP, 2], [2, 2], [1, 512]]),
                    ap4(u2, 0, [[512, 2], [0, 2], [1, 512]]))
                if chunked:
                    gelu_chunk(0, 1024)
                nc.vector.tensor_add(
                    ap4(tmp, 1024, [[5120, 2], [512, 2], [1, 512]]),
                    ap4(vev2, PADL + 2, [[TP, 2], [2, 2], [1, 512]]),
                    ap4(u2, 0, [[512, 2], [0, 2], [1, 512]]))
                if chunked:
                    gelu_chunk(1024, 1024)
                    nc.vector.tensor_add(
                        ap4(tmp, 2048, [[5120, 2], [512, 3], [1, 512]]),
                        ap4(vod2, PADL - 6, [[TP, 2], [2, 3], [1, 512]]),
                        ap4(u2, 0, [[512, 2], [0, 3], [1, 512]]))
                    gelu_chunk(2048, 1536)
                    nc.vector.tensor_add(
                        ap4(tmp, 3584, [[5120, 2], [512, 3], [1, 512]]),
                        ap4(vod2, PADL, [[TP, 2], [2, 3], [1, 512]]),
                        ap4(u2, 0, [[512, 2], [0, 3], [1, 512]]))
                    gelu_chunk(3584, 1536)
                    return
                nc.vector.tensor_add(
                    ap4(tmp, 2048, [[5120, 2], [512, 6], [1, 512]]),
                    ap4(vod2, PADL - 6, [[TP, 2], [2, 6], [1, 512]]),
                    ap4(u2, 0, [[512, 2], [0, 6], [1, 512]]))
                if emit_gelu:
                    stage_d_gelu(p)

            def stage_d_combine(p):
                tmp = tmp_sb[p]
                # cw multiply in place (elementwise same-position or
                # write-behind -> safe), broadcast across the pair via a
                # step-0 fc dim on cw_bc.  cw_bc is in sorted-offset order
                # (11 blocks); tmp's per-phase block groups map to uniform
                # stride-2 block runs, so 3 instructions cover it.
                nc.vector.tensor_mul(
                    ap4(tmp, 0, [[5120, 2], [512, 2], [1, 512]]),
                    ap4(tmp, 0, [[5120, 2], [512, 2], [1, 512]]),
                    ap4(cw_bc, 1 * 512, [[0, 2], [1024, 2], [1, 512]]))
                nc.vector.tensor_mul(
                    ap4(tmp, 1024, [[5120, 2], [512, 2], [1, 512]]),
                    ap4(tmp, 1024, [[5120, 2], [512, 2], [1, 512]]),
                    ap4(cw_bc, 7 * 512, [[0, 2], [1024, 2], [1, 512]]))
                nc.vector.tensor_mul(
                    ap4(tmp, 2048, [[5120, 2], [512, 6], [1, 512]]),
                    ap4(tmp, 2048, [[5120, 2], [512, 6], [1, 512]]),
                    ap4(cw_bc, 0, [[0, 2], [1024, 6], [1, 512]]))
                # pairwise tree-sum of the 10 weighted slices per half
                t1 = qpool.tile([128, 5120], bf16, tag="t1")
                nc.vector.tensor_add(
                    t1[:],
                    ap4(tmp, 0, [[5120, 2], [1, 2560]]),
                    ap4(tmp, 2560, [[5120, 2], [1, 2560]]))
                t2 = qpool.tile([128, 2048], bf16, tag="t2")
                nc.vector.tensor_add(
                    t2[:],
                    ap4(t1, 0, [[2560, 2], [1, 1024]]),
                    ap4(t1, 1024, [[2560, 2], [1, 1024]]))
                t3 = qpool.tile([128, 1024], bf16, tag="t3")
                nc.vector.tensor_add(
                    t3[:],
                    ap4(t2, 0, [[1024, 2], [1, 512]]),
                    ap4(t2, 512, [[1024, 2], [1, 512]]))
                t4 = qpool.tile([128, 1024], bf16, tag="t4")
                nc.vector.tensor_add(
                    t4[:], t3[:],
                    ap4(t1, 2048, [[2560, 2], [1, 512]]))
                g2 = gpool.tile([128, 1024], bf16, tag=f"g{p}")
                nc.vector.tensor_mul(
                    g2[:], t4[:],
                    ap4(w_bc_all, p * 512, [[0, 2], [1, 512]]))
                g_sb[2 * p] = g2[:, 0:512]
                g_sb[2 * p + 1] = g2[:, 512:1024]

            w2_sb = [None] * NFC

            def load_w2(j):
                t = w2pool.tile([128, D], bf16, tag=f"w2_{j}")
                nc.sync.dma_start(t[:], w2[j])
                w2_sb[j] = t

            def blk_mm(d_ps, blk, fc, start, stop=False):
                tci, dh = blk // 2, blk % 2
                nc.tensor.matmul(
                    d_ps[:],
                    g_sb[fc][:, tci * 128:(tci + 1) * 128],
                    w2_sb[fc][:, dh * 512:(dh + 1) * 512],
                    start=start, stop=stop)


            def blk_b2_mm(d_ps, blk):
                tci, dh = blk // 2, blk % 2
                nc.tensor.matmul(
                    d_ps[:],
                    wT_bf[:, tci * 128:(tci + 1) * 128],
                    b2_sb[:, dh * 512:(dh + 1) * 512],
                    start=False, stop=True)

            def out_dma(o_sb, blk):
                tci, dh = blk // 2, blk % 2
                nc.sync.dma_start(
                    out[tci * 128:(tci + 1) * 128,
                        dh * 512:(dh + 1) * 512], o_sb[:])

            # -- delta groups.  Held groups (blocks 0..2 on "s" banks, and
            # preopened blocks 3..6 on freed "m" banks) accumulate fc matmuls
            # per combine and close with b2 + a per-token 1/den scale.
            open_ps = {}

            def grp_open(blk, g_lo, g_hi, pool):
                d_ps = pool.tile([128, 512],
                                 mybir.dt.float32, tag="m" if pool is psb
                                 else "s", name=f"dps{blk}")
                for fc in range(g_lo, g_hi + 1):
                    blk_mm(d_ps, blk, fc, start=(fc == g_lo))
                open_ps[blk] = d_ps

            def grp_extend(blk, fc):
                blk_mm(open_ps[blk], blk, fc, start=False)

            def grp_close_direct(blk):
                """For groups that accumulated all of fc 0..15."""
                tci = blk // 2
                d_ps = open_ps[blk]
                blk_b2_mm(d_ps, blk)
                o_sb = opool.tile([128, 512], fp32, tag="o")
                nc.scalar.mul(o_sb[:], d_ps[:], rdenT[:, tci:tci + 1])
                out_dma(o_sb, blk)

            d_part = {}

            def stage_e1(blk):
                """fc 0..7 partial for blocks 3..7 (one pss bank transient)."""
                tci = blk // 2
                d_ps = pss.tile([128, 512], fp32, tag="s")
                for fc in range(8):
                    blk_mm(d_ps, blk, fc, start=(fc == 0), stop=(fc == 7))
                p_t = partpool.tile([128, 512], bf16, tag=f"p{blk}")
                nc.scalar.mul(p_t[:], d_ps[:], rdenT[:, tci:tci + 1])
                d_part[blk] = p_t

            def grp_close_merge(blk):
                """For groups that accumulated fc 8..15: merge with the E1
                partial via one scalar_tensor_tensor."""
                tci = blk // 2
                d_ps = open_ps[blk]
                blk_b2_mm(d_ps, blk)
                o_sb = opool.tile([128, 512], fp32, tag="o")
                nc.vector.scalar_tensor_tensor(
                    o_sb[:], d_ps[:], rdenT[:, tci:tci + 1], d_part[blk][:],
                    op0=OP.mult, op1=OP.add)
                out_dma(o_sb, blk)

            # ---- emission schedule ----
            # Score path first: all 4 gram tiles + extracts, then the full
            # B phase (transposes, exps, cw/w DRAM round trips) BEFORE any
            # gelu, so (a) the extracts aren't priority-starved by adds,
            # (b) the exps share one ACT table residency, (c) the cw/w
            # broadcasts are in flight by ~20us.  The ready-heap scheduler
            # backfills PE with d-half matmuls while the extract chain and
            # b-phase drain on Vector/ACT.
            phase_gram(0)
            phase_gram(1)
            phase_gram(2)
            phase_gram(3)
            phase_b1()
            phase_b2a_pre()
            phase_b2a_post()
            phase_b2b_early()
            stage_d_half(0, w1_pre=w1_first)
            stage_d_half(1)
            stage_d_adds(0, chunked=True)
            phase_b2b_late()
            stage_d_half(2)
            stage_d_half(3)
            stage_d_adds(1, chunked=True)
            for p in range(NP):             # pair-combine index
                pp = p + 2
                if pp < NP:
                    stage_d_half(2 * pp)
                    stage_d_half(2 * pp + 1)
                    stage_d_adds(pp)
                    if p < 4:
                        for jw in range(4 * p, 4 * p + 4):
                            load_w2(jw)
                stage_d_combine(p)
                if p == 3:
                    stage_e1(3)
                if p == 4:
                    stage_e1(4)
                    stage_e1(5)
                if p == 5:
                    stage_e1(6)
                    stage_e1(7)
                    grp_open(0, 0, 11, pss)
                if p == 6:
                    grp_extend(0, 12)
                    grp_extend(0, 13)
                    grp_open(1, 0, 13, pss)
                    grp_open(2, 0, 13, pss)
                    for blk in range(3, 7):
                        grp_open(blk, 8, 13, psb)
            # tail: g14/g15 + b2 per open group, then block 7 full
            for blk in (0, 1, 2, 3, 4, 5, 6):
                grp_extend(blk, 14)
                grp_extend(blk, 15)
            grp_close_direct(0)
            grp_open(7, 8, 15, pss)
            grp_close_direct(1)
            grp_close_direct(2)
            for blk in (3, 4, 5, 6):
                grp_close_merge(blk)
            grp_close_merge(7)

    nc.compile()
    return nc


def _prep_shards(h_L, mask_flags, Wr, br, W1, b1, W2, b2):
    """Host-side shard construction (numpy only; cheap vs device work)."""
    f32 = np.float32
    h_L = np.asarray(h_L, f32)
    mask = np.asarray(mask_flags)
    Wr = np.asarray(Wr, f32)
    W1 = np.asarray(W1, f32)
    W2 = np.asarray(W2, f32)
    br = np.asarray(br, f32)
    b1 = np.asarray(b1, f32)
    b2 = np.asarray(b2, f32)

    # shared (replicated) weight blocks
    w1a = np.ascontiguousarray(
        W1[:, :D, :].transpose(1, 0, 2).reshape(D, F)
        .reshape(NKC, 128, NFC, 128).transpose(2, 1, 0, 3)
        .reshape(NFC, 128, D)).astype(BF16)
    w1b = np.ascontiguousarray(
        W1[:, D:, :].transpose(1, 0, 2).reshape(D, F)
        .reshape(NKC, 128, NFC, 128).transpose(2, 1, 0, 3)
        .reshape(NFC, 128, D)).astype(BF16)
    w1ab = np.concatenate([w1b, w1a], axis=2)        # [NFC, 128, 2D]
    w2p = np.ascontiguousarray(
        W2.reshape(F, D).reshape(NFC, 128, D)).astype(BF16)
    # packed wr: [128, NKC*K] + a trailing ones column (den ones-matmul)
    wrP = np.concatenate([
        np.ascontiguousarray(
            Wr.reshape(NKC, 128, K).transpose(1, 0, 2).reshape(128, NKC * K)
        ), np.ones((128, 1), np.float32)], axis=1).astype(BF16)
    # packed consts: ident | br_bc | b1s  -> [128, 176] fp32
    br_bc = np.tile(np.broadcast_to(br[None, :], (128, K)), (1, NTC)).astype(f32)
    b1s = np.ascontiguousarray(b1.reshape(F).reshape(NFC, 128).T)
    cP = np.concatenate([np.eye(128, dtype=f32), br_bc, b1s], axis=1)
    # packed b2 + ones column: [NOFF, D+1] bf16
    b2o = np.zeros((NOFF, D + 2), BF16)
    b2o[:K, :D] = b2.astype(BF16)
    b2o[:, D] = 1.0

    in_maps = []
    outs_meta = []
    per_batch = L // (NCORES // B)          # 512 tokens, 4 shards per batch
    for c in range(NCORES):
        b = c // (NCORES // B)
        t0 = (c % (NCORES // B)) * per_batch
        # padded, transposed h slice  [D, TP] -> packed [128, NKC*TP]
        hpad = np.zeros((TP, D), f32)
        lo = t0 - PADL
        hi = t0 + T + PADL
        slo, shi = max(lo, 0), min(hi, L)
        hpad[slo - lo:shi - lo] = h_L[b, slo:shi]
        hTa = np.ascontiguousarray(hpad.T).astype(BF16)          # [D, TP]
        hP = np.ascontiguousarray(
            hTa.reshape(NKC, 128, TP).transpose(1, 0, 2)
            .reshape(128, NKC * TP))

        # validity per (offset SORTED -5..+5 incl dead center, token)
        tok = t0 + np.arange(T)
        offs11 = np.arange(-R, R + 1)
        nbr = tok[:, None] + offs11[None, :]
        inb = (nbr >= 0) & (nbr < L)
        nbrc = np.clip(nbr, 0, L - 1)
        is_m = (mask[b] == 1)
        val = (inb & is_m[tok][:, None] & (~is_m[nbrc]))
        val[:, R] = False                             # center: never valid
        valT = np.ascontiguousarray(val.T.astype(np.float32)).astype(BF16)
        cPc = np.concatenate(
            [cP, np.zeros((128, 40), f32)], axis=1)
        in_maps.append({
            "hP": hP, "cP": cPc, "wrP": wrP, "validT": valT, "b2o": b2o,
            "w1ab": w1ab, "w2": w2p,
        })
        outs_meta.append((b, t0))
    return in_maps, outs_meta


def kernel(**inputs):
    assert int(inputs["range_r"]) == R
    if "nc" not in _CACHE:
        _CACHE["nc"] = _build_graph()
    nc = _CACHE["nc"]
    in_maps, outs_meta = _prep_shards(
        inputs["h_L"], inputs["mask_flags"], inputs["Wr"], inputs["br"],
        inputs["W1"], inputs["b1"], inputs["W2"], inputs["b2"])
    res = run_bass_kernel_spmd(nc, in_maps, core_ids=list(range(NCORES)))
    out = np.zeros((B, L, D), np.float32)
    for c, (b, t0) in enumerate(outs_meta):
        out[b, t0:t0 + T] = res.results[c]["out"]
    return out

